# revision 6
# baseline (speedup 1.0000x reference)
"""Trainium2 Bass kernel for a 16-layer fully-connected chain (matvec per layer).

Computation (reference):
    v = x                       # [2048]
    for i in 0..13:  v = silu(W[i] @ v + b[i])
    out = W[14] @ v + b[14]

Design (8 NeuronCores, one trn2 chip):
  - Layer 0 is computed REDUNDANTLY in full on every core (256 matmul pairs,
    ~7 us, and its 8 MB fp16 weight DMA) -- both hide entirely inside the
    ~50 us ncfw collective-init barrier that stalls the first AllGather
    anyway, eliminating one gather round.
  - Layers 1..14 are row-sharded: core r computes output neurons
    [r*256, (r+1)*256) as two m-tiles of 128, outputs living ACROSS the 128
    partitions (weights-stationary matmuls: lhsT = W^T tile [k=128, m=128],
    rhs = activation column [128,1], fp32 PSUM accumulated over 16 k-tiles).
    Biases are folded into PSUM with a rank-1 matmul (lhsT = bias row
    [1,128], rhs = const 1.0), so the epilogue is one [128,2] sigmoid + one
    tensor-multiply.
  - fp16 weights+activations with per-layer power-of-2 activation scaling
    folded into the weights host-side (activations grow ~3.2x/layer to ~3e7,
    far beyond fp16 range): psum = y_i/S[i+1], sg = sigmoid(S[i+1]*psum),
    v' = psum*sg (silu is not scale-equivariant). Final layer:
    out = S[14]*psum in fp32.
  - All weight slices are DMA'd to SBUF up front and stream behind compute.
  - Inter-layer gather: the [128, 2] half-pair is PE-transposed to [2, 128]
    so every comm DMA is per-partition contiguous (a [128, few-bytes] DMA
    shatters into per-partition 4B packets, ~13 us/layer of queue drain);
    one fp16 AllGather (512 B -> 4 KB) per layer through internal DRAM
    bounce buffers, then a [16,128] -> [128,16] PE transpose back.
"""

import numpy as np

_L = 15        # number of weight matrices
_N = 2048      # neurons per layer
_M = 8         # cores
_SH = _N // _M  # 256 outputs per core = 2 m-tiles of 128

# S[i] = scale of the stored activation entering layer i (v_i = S[i]*v'_i).
_S = [1.0, 0.5, 1.0, 4.0, 8.0, 32.0, 128.0, 256.0, 1024.0, 2048.0,
      8192.0, 16384.0, 65536.0, 262144.0, 524288.0]

# bias_mm column bases (x128): layers 2..14 pairs first, then layers 0/1 full.
_NBIAS = 26 + 32
_ONES_COL = _NBIAS * 128

_CACHE = {}


def _build():
    import concourse.bacc as bacc
    import concourse.mybir as mybir
    import concourse.tile as tile

    f32 = mybir.dt.float32
    f16 = mybir.dt.float16
    AF = mybir.ActivationFunctionType

    nc = bacc.Bacc("TRN2", target_bir_lowering=False, debug=False,
                   num_devices=_M)

    # full layer-0/1 weights: col (kt*16 + mo)*128 + pm holds
    # scaled W[f][mo*128+pm, kt*128+pk] at partition pk.
    wtf = nc.dram_tensor("wtf", [2, 128, 256 * 128], f16,
                         kind="ExternalInput")
    # sharded layers 2..14: per layer, col ((mt*8 + j)*2 + mo)*128 + pm holds
    # W[i][r*256+mo*128+pm, j*256+mt*128+pk] at partition pk (pre-scaled).
    wt = nc.dram_tensor("wt", [_L - 2, 128, 32 * 128], f16,
                        kind="ExternalInput")
    ident = nc.dram_tensor("ident", [128, 128], f16, kind="ExternalInput")
    x0 = nc.dram_tensor("x0", [128, 16], f16, kind="ExternalInput")
    bias_mm = nc.dram_tensor("bias_mm", [1, _NBIAS * 128 + 1], f16,
                             kind="ExternalInput")
    out = nc.dram_tensor("out", [128, 2], f32, kind="ExternalOutput")

    with tile.TileContext(nc) as tc:
        with (
            tc.tile_pool(name="w0", bufs=1) as w0pool,
            tc.tile_pool(name="w", bufs=_L - 2) as wpool,
            tc.tile_pool(name="g", bufs=_L) as gpool,
            tc.tile_pool(name="src", bufs=4) as srcpool,
            tc.tile_pool(name="sg", bufs=4) as sgpool,
            tc.tile_pool(name="c", bufs=1) as cpool,
            tc.tile_pool(name="ps", bufs=2, space="PSUM") as pspool,
            tc.tile_pool(name="psf", bufs=2, space="PSUM") as psfpool,
            tc.tile_pool(name="pst", bufs=2, space="PSUM") as pstpool,
            tc.tile_pool(name="pgt", bufs=2, space="PSUM") as pgtpool,
            tc.tile_pool(name="dram", bufs=2 * _L, space="DRAM") as dpool,
        ):
            bias_t = cpool.tile([1, _NBIAS * 128 + 1], f16)
            nc.sync.dma_start(bias_t[:], bias_mm.ap())
            ident_t = cpool.tile([128, 128], f16)
            nc.sync.dma_start(ident_t[:], ident.ap())

            gath = [gpool.tile([128, 16], f16, tag="g", name=f"gath{i}")
                    for i in range(_L)]
            nc.sync.dma_start(gath[0][:], x0.ap())

            wsb = [None, None]
            for i in range(2, _L):
                w = wpool.tile([128, 32 * 128], f16, tag="w")
                nc.sync.dma_start(w[:], wt.ap()[i - 2])
                wsb.append(w)

            ones = bias_t[:, _ONES_COL:_ONES_COL + 1]

            # ---- layers 0 and 1: full 2048x2048 matvec on every core
            # (redundant compute + their 8 MB weight DMAs hide inside the
            # ncfw collective-init barrier window). One start/stop pair per
            # 2KB PSUM zero region: start=True zeroes the WHOLE region, so
            # only the very first matmul starts and only the very last stops.
            for f in range(2):
                wfsb = w0pool.tile([128, 256 * 128], f16, tag="wf",
                                   name=f"wf{f}")
                nc.sync.dma_start(wfsb[:], wtf.ap()[f])
                psf = psfpool.tile([128, 16], f32, tag="psf",
                                   name=f"psf{f}")
                for kt in range(16):
                    for mo in range(16):
                        c = kt * 16 + mo
                        nc.tensor.matmul(
                            psf[:, mo:mo + 1],
                            lhsT=wfsb[:, c * 128:(c + 1) * 128],
                            rhs=gath[f][:, kt:kt + 1],
                            start=(kt == 0 and mo == 0), stop=False,
                        )
                for mo in range(16):
                    c0 = (26 + 16 * f + mo) * 128
                    nc.tensor.matmul(
                        psf[:, mo:mo + 1],
                        lhsT=bias_t[:, c0:c0 + 128],
                        rhs=ones,
                        start=False, stop=(mo == 15),
                    )
                sgf = sgpool.tile([128, 16], f32, tag="sgf")
                nc.scalar.activation(sgf[:], psf[:], AF.Sigmoid,
                                     scale=float(_S[f + 1]))
                nc.vector.tensor_mul(gath[f + 1][:], psf[:], sgf[:])

            # ---- layers 2..14: row-sharded with AllGather between ----
            for i in range(2, _L):
                ps = pspool.tile([128, 2], f32, tag="ps", name=f"ps{i}")
                for mt in range(2):
                    for j in range(_M):
                        for mo in range(2):
                            c = (mt * 8 + j) * 2 + mo
                            nc.tensor.matmul(
                                ps[:, mo:mo + 1],
                                lhsT=wsb[i][:, c * 128:(c + 1) * 128],
                                rhs=gath[i][:, 2 * j + mt:2 * j + mt + 1],
                                start=(mt == 0 and j == 0 and mo == 0),
                                stop=False,
                            )
                for mo in range(2):
                    c0 = ((i - 2) * 2 + mo) * 128
                    nc.tensor.matmul(
                        ps[:, mo:mo + 1],
                        lhsT=bias_t[:, c0:c0 + 128],
                        rhs=ones,
                        start=False, stop=(mo == 1),
                    )
                if i < _L - 1:
                    src = srcpool.tile([128, 2], f16, tag="src")
                    sg = sgpool.tile([128, 2], f32, tag="sg")
                    # psum = y_i/S[i+1] (bias folded); sg = sigmoid(y_i)
                    nc.scalar.activation(sg[:], ps[:], AF.Sigmoid,
                                         scale=float(_S[i + 1]))
                    nc.vector.tensor_mul(src[:], ps[:], sg[:])
                    # PE-transpose [128,2] -> [2,128] so comm DMAs stay
                    # per-partition contiguous.
                    srcT = pstpool.tile([2, 128], f16, tag="pst",
                                        name=f"srcT{i}")
                    nc.tensor.transpose(srcT[:], src[:], ident_t[:])
                    srcTs = srcpool.tile([2, 128], f16, tag="srcTs")
                    nc.vector.tensor_copy(srcTs[:], srcT[:])
                    cc_in = dpool.tile([2, 128], f16, tag="ccin")
                    nc.scalar.dma_start(cc_in[:], srcTs[:])
                    cc_out = dpool.tile([16, 128], f16, tag="ccout")
                    nc.gpsimd.collective_compute(
                        "AllGather",
                        mybir.AluOpType.bypass,
                        replica_groups=[list(range(_M))],
                        ins=[cc_in.opt()],
                        outs=[cc_out.opt()],
                    )
                    rdraw = srcpool.tile([16, 128], f16, tag="rdraw")
                    nc.scalar.dma_start(rdraw[:], cc_out[:])
                    gT = pgtpool.tile([128, 16], f16, tag="gT",
                                      name=f"gT{i}")
                    nc.tensor.transpose(gT[:], rdraw[:], ident_t[:16, :16])
                    nc.vector.tensor_copy(gath[i + 1][:], gT[:])
                else:
                    o = srcpool.tile([128, 2], f32, tag="o")
                    # out = S[14] * psum  (bias already folded into psum)
                    nc.scalar.activation(o[:], ps[:], AF.Identity,
                                         scale=float(_S[14]))
                    nc.sync.dma_start(out.ap(), o[:])

    nc.compile()
    return nc


def _prep_inputs(x, W, b):
    """Host-side scaling, transposition and per-core slicing."""
    x = np.asarray(x, np.float32)
    W = np.asarray(W, np.float32)
    b = np.asarray(b, np.float32)
    S = _S

    Wf = np.empty_like(W)
    for i in range(_L - 1):
        Wf[i] = W[i] * (S[i] / S[i + 1])
    Wf[_L - 1] = W[_L - 1]  # folded with S15 = S14

    # layers 0/1 full: [pk, kt, mo, pm] -> col (kt*16+mo)*128+pm
    Wfl = Wf[:2].reshape(2, 16, 128, 16, 128)   # [f, mo, pm, kt, pk]
    wtf = np.ascontiguousarray(
        Wfl.transpose(0, 4, 3, 1, 2).reshape(2, 128, 256 * 128)
    ).astype(np.float16)

    # layers 2..14 sharded: Wv[i, rm, mo, pm, ks, mt, pk]
    Wv = Wf[2:].reshape(_L - 2, _M, 2, 128, _M, 2, 128)
    xv = x.reshape(_M, 2, 128)
    # x0[pk, 2j+mt] = x[j*256 + mt*128 + pk]
    x0 = np.ascontiguousarray(
        xv.transpose(2, 0, 1).reshape(128, 16)).astype(np.float16)
    identity = np.eye(128, dtype=np.float16)

    in_maps = []
    for r in range(_M):
        Wc = Wv[:, r]                           # [i, mo, pm, j, mt, pk]
        Wc = Wc.transpose(0, 5, 4, 3, 1, 2)     # [i, pk, mt, j, mo, pm]
        wt_r = np.ascontiguousarray(
            Wc.reshape(_L - 2, 128, 32 * 128)).astype(np.float16)
        # bias_mm: layers 2..14 pairs, then layers 0/1 full, then 1.0
        bias = np.zeros(_NBIAS * 128 + 1, np.float32)
        for i in range(2, _L):
            s = S[i + 1] if i < _L - 1 else S[_L - 1]
            for mo in range(2):
                c0 = ((i - 2) * 2 + mo) * 128
                bias[c0:c0 + 128] = b[i, r * 256 + mo * 128:
                                      r * 256 + (mo + 1) * 128] / s
        for f in range(2):
            for mo in range(16):
                c0 = (26 + 16 * f + mo) * 128
                bias[c0:c0 + 128] = b[f, mo * 128:(mo + 1) * 128] / S[f + 1]
        bias[_ONES_COL] = 1.0
        in_maps.append({"wtf": wtf, "wt": wt_r, "x0": x0, "ident": identity,
                        "bias_mm": bias.reshape(1, -1).astype(np.float16)})
    return in_maps


def kernel(x, W, b, _trace=False):
    from concourse.bass_utils import run_bass_kernel_spmd

    key = "nc"
    if key not in _CACHE:
        _CACHE[key] = _build()
    nc = _CACHE[key]

    in_maps = _prep_inputs(x, W, b)
    res = run_bass_kernel_spmd(
        nc, in_maps, core_ids=list(range(_M)), trace=_trace)
    _CACHE["last_results"] = res
    return np.concatenate(
        [res.results[r]["out"].T.reshape(_SH) for r in range(_M)])


# revision 9
# speedup vs baseline: 1.0293x; 1.0293x over previous
"""Trainium2 Bass kernel for a 16-layer fully-connected chain (matvec per layer).

Computation (reference):
    v = x                       # [2048]
    for i in 0..13:  v = silu(W[i] @ v + b[i])
    out = W[14] @ v + b[14]

Design (8 NeuronCores, one trn2 chip):
  - Layer 0 is computed REDUNDANTLY in full on every core (256 matmul pairs,
    ~7 us, and its 8 MB fp16 weight DMA) -- both hide entirely inside the
    ~50 us ncfw collective-init barrier that stalls the first AllGather
    anyway, eliminating one gather round.
  - Layers 1..14 are row-sharded: core r computes output neurons
    [r*256, (r+1)*256) as two m-tiles of 128, outputs living ACROSS the 128
    partitions (weights-stationary matmuls: lhsT = W^T tile [k=128, m=128],
    rhs = activation column [128,1], fp32 PSUM accumulated over 16 k-tiles).
    Biases are folded into PSUM with a rank-1 matmul (lhsT = bias row
    [1,128], rhs = const 1.0), so the epilogue is one [128,2] sigmoid + one
    tensor-multiply.
  - fp16 weights+activations with per-layer power-of-2 activation scaling
    folded into the weights host-side (activations grow ~3.2x/layer to ~3e7,
    far beyond fp16 range): psum = y_i/S[i+1], sg = sigmoid(S[i+1]*psum),
    v' = psum*sg (silu is not scale-equivariant). Final layer:
    out = S[14]*psum in fp32.
  - All weight slices are DMA'd to SBUF up front and stream behind compute.
  - Inter-layer gather: the [128, 2] half-pair is PE-transposed to [2, 128]
    so every comm DMA is per-partition contiguous (a [128, few-bytes] DMA
    shatters into per-partition 4B packets, ~13 us/layer of queue drain);
    one fp16 AllGather (512 B -> 4 KB) per layer through internal DRAM
    bounce buffers, then a [16,128] -> [128,16] PE transpose back.
"""

import numpy as np

_L = 15        # number of weight matrices
_N = 2048      # neurons per layer
_M = 8         # cores
_SH = _N // _M  # 256 outputs per core = 2 m-tiles of 128

# S[i] = scale of the stored activation entering layer i (v_i = S[i]*v'_i).
_S = [1.0, 0.5, 1.0, 4.0, 8.0, 32.0, 128.0, 256.0, 1024.0, 2048.0,
      8192.0, 16384.0, 65536.0, 262144.0, 524288.0]

# bias_mm column bases (x128): layers 1..14 pairs first, then layer-0 full.
_NBIAS = 28 + 16
_ONES_COL = _NBIAS * 128

_CACHE = {}


def _build():
    import concourse.bacc as bacc
    import concourse.mybir as mybir
    import concourse.tile as tile

    f32 = mybir.dt.float32
    f16 = mybir.dt.float16
    AF = mybir.ActivationFunctionType

    nc = bacc.Bacc("TRN2", target_bir_lowering=False, debug=False,
                   num_devices=_M)

    # full layer-0 weights: col (kt*16 + mo)*128 + pm holds
    # 2*W[0][mo*128+pm, kt*128+pk] at partition pk.
    wt0 = nc.dram_tensor("wt0", [128, 256 * 128], f16, kind="ExternalInput")
    # sharded layers 1..14: per layer, col ((mt*8 + j)*2 + mo)*128 + pm holds
    # W[i][r*256+mo*128+pm, j*256+mt*128+pk] at partition pk (pre-scaled).
    wt = nc.dram_tensor("wt", [_L - 1, 128, 32 * 128], f16,
                        kind="ExternalInput")
    ident = nc.dram_tensor("ident", [128, 128], f16, kind="ExternalInput")
    x0 = nc.dram_tensor("x0", [128, 16], f16, kind="ExternalInput")
    bias_mm = nc.dram_tensor("bias_mm", [1, _NBIAS * 128 + 1], f16,
                             kind="ExternalInput")
    out = nc.dram_tensor("out", [128, 2], f32, kind="ExternalOutput")

    with tile.TileContext(nc) as tc:
        with (
            tc.tile_pool(name="w0", bufs=1) as w0pool,
            tc.tile_pool(name="w", bufs=_L - 1) as wpool,
            tc.tile_pool(name="g", bufs=_L) as gpool,
            tc.tile_pool(name="src", bufs=4) as srcpool,
            tc.tile_pool(name="sg", bufs=4) as sgpool,
            tc.tile_pool(name="c", bufs=1) as cpool,
            tc.tile_pool(name="ps", bufs=3, space="PSUM") as pspool,
            tc.tile_pool(name="psf", bufs=1, space="PSUM") as psfpool,
            tc.tile_pool(name="pst", bufs=2, space="PSUM") as pstpool,
            tc.tile_pool(name="pgt", bufs=2, space="PSUM") as pgtpool,
            tc.tile_pool(name="dram", bufs=2 * _L, space="DRAM") as dpool,
        ):
            bias_t = cpool.tile([1, _NBIAS * 128 + 1], f16)
            nc.sync.dma_start(bias_t[:], bias_mm.ap())
            ident_t = cpool.tile([128, 128], f16)
            nc.sync.dma_start(ident_t[:], ident.ap())

            gath = [gpool.tile([128, 16], f16, tag="g", name=f"gath{i}")
                    for i in range(_L)]
            nc.sync.dma_start(gath[0][:], x0.ap())

            w0sb = w0pool.tile([128, 256 * 128], f16)
            nc.sync.dma_start(w0sb[:], wt0.ap())
            wsb = [None]
            for i in range(1, _L):
                w = wpool.tile([128, 32 * 128], f16, tag="w")
                nc.sync.dma_start(w[:], wt.ap()[i - 1])
                wsb.append(w)

            ones = bias_t[:, _ONES_COL:_ONES_COL + 1]

            # ---- layer 0: full 2048x2048 matvec on every core ----
            # One start/stop pair per 2KB PSUM zero region: start=True zeroes
            # the WHOLE region, so only the very first matmul starts and only
            # the very last stops; per-address first-touch semantics handle
            # the other columns.
            psf = psfpool.tile([128, 16], f32)
            for kt in range(16):
                for mo in range(16):
                    c = kt * 16 + mo
                    nc.tensor.matmul(
                        psf[:, mo:mo + 1],
                        lhsT=w0sb[:, c * 128:(c + 1) * 128],
                        rhs=gath[0][:, kt:kt + 1],
                        start=(kt == 0 and mo == 0), stop=False,
                    )
            for mo in range(16):
                c0 = (28 + mo) * 128
                nc.tensor.matmul(
                    psf[:, mo:mo + 1],
                    lhsT=bias_t[:, c0:c0 + 128],
                    rhs=ones,
                    start=False, stop=(mo == 15),
                )
            sgf = sgpool.tile([128, 16], f32, tag="sgf")
            nc.scalar.activation(sgf[:], psf[:], AF.Sigmoid,
                                 scale=float(_S[1]))
            nc.vector.tensor_mul(gath[1][:], psf[:], sgf[:])

            # ---- layers 1..14: row-sharded with AllGather between ----
            for i in range(1, _L):
                ps = pspool.tile([128, 2], f32, tag="ps", name=f"ps{i}")
                for mt in range(2):
                    for j in range(_M):
                        for mo in range(2):
                            c = (mt * 8 + j) * 2 + mo
                            nc.tensor.matmul(
                                ps[:, mo:mo + 1],
                                lhsT=wsb[i][:, c * 128:(c + 1) * 128],
                                rhs=gath[i][:, 2 * j + mt:2 * j + mt + 1],
                                start=(mt == 0 and j == 0 and mo == 0),
                                stop=False,
                            )
                for mo in range(2):
                    c0 = ((i - 1) * 2 + mo) * 128
                    nc.tensor.matmul(
                        ps[:, mo:mo + 1],
                        lhsT=bias_t[:, c0:c0 + 128],
                        rhs=ones,
                        start=False, stop=(mo == 1),
                    )
                if i < _L - 1:
                    src = srcpool.tile([128, 2], f16, tag="src")
                    sg = sgpool.tile([128, 2], f32, tag="sg")
                    # psum = y_i/S[i+1] (bias folded); sg = sigmoid(y_i)
                    nc.scalar.activation(sg[:], ps[:], AF.Sigmoid,
                                         scale=float(_S[i + 1]))
                    nc.vector.tensor_mul(src[:], ps[:], sg[:])
                    # PE-transpose [128,2] -> [2,128] so comm DMAs stay
                    # per-partition contiguous.
                    srcT = pstpool.tile([2, 128], f16, tag="pst",
                                        name=f"srcT{i}")
                    nc.tensor.transpose(srcT[:], src[:], ident_t[:])
                    srcTs = srcpool.tile([2, 128], f16, tag="srcTs")
                    nc.vector.tensor_copy(srcTs[:], srcT[:])
                    cc_in = dpool.tile([2, 128], f16, tag="ccin")
                    nc.scalar.dma_start(cc_in[:], srcTs[:])
                    cc_out = dpool.tile([16, 128], f16, tag="ccout")
                    nc.gpsimd.collective_compute(
                        "AllGather",
                        mybir.AluOpType.bypass,
                        replica_groups=[list(range(_M))],
                        ins=[cc_in.opt()],
                        outs=[cc_out.opt()],
                    )
                    rdraw = srcpool.tile([16, 128], f16, tag="rdraw")
                    nc.scalar.dma_start(rdraw[:], cc_out[:])
                    gT = pgtpool.tile([128, 16], f16, tag="gT",
                                      name=f"gT{i}")
                    nc.tensor.transpose(gT[:], rdraw[:], ident_t[:16, :16])
                    nc.vector.tensor_copy(gath[i + 1][:], gT[:])
                else:
                    o = srcpool.tile([128, 2], f32, tag="o")
                    # out = S[14] * psum  (bias already folded into psum)
                    nc.scalar.activation(o[:], ps[:], AF.Identity,
                                         scale=float(_S[14]))
                    nc.sync.dma_start(out.ap(), o[:])

    nc.compile()
    return nc


def _prep_inputs(x, W, b):
    """Host-side scaling, transposition and per-core slicing."""
    x = np.asarray(x, np.float32)
    W = np.asarray(W, np.float32)
    b = np.asarray(b, np.float32)
    S = _S

    Wf = np.empty_like(W)
    for i in range(_L - 1):
        Wf[i] = W[i] * (S[i] / S[i + 1])
    Wf[_L - 1] = W[_L - 1]  # folded with S15 = S14

    # layer-0 full: [pk, kt, mo, pm] -> col (kt*16+mo)*128+pm
    W0 = Wf[0].reshape(16, 128, 16, 128)        # [mo, pm, kt, pk]
    wt0 = np.ascontiguousarray(
        W0.transpose(3, 2, 0, 1).reshape(128, 256 * 128)).astype(np.float16)

    # layers 1..14 sharded: Wv[i, rm, mo, pm, ks, mt, pk]
    Wv = Wf[1:].reshape(_L - 1, _M, 2, 128, _M, 2, 128)
    xv = x.reshape(_M, 2, 128)
    # x0[pk, 2j+mt] = x[j*256 + mt*128 + pk]
    x0 = np.ascontiguousarray(
        xv.transpose(2, 0, 1).reshape(128, 16)).astype(np.float16)
    identity = np.eye(128, dtype=np.float16)

    in_maps = []
    for r in range(_M):
        Wc = Wv[:, r]                           # [i, mo, pm, j, mt, pk]
        Wc = Wc.transpose(0, 5, 4, 3, 1, 2)     # [i, pk, mt, j, mo, pm]
        wt_r = np.ascontiguousarray(
            Wc.reshape(_L - 1, 128, 32 * 128)).astype(np.float16)
        # bias_mm: layers 1..14 pairs, then layer-0 full, then 1.0
        bias = np.zeros(_NBIAS * 128 + 1, np.float32)
        for i in range(1, _L):
            s = S[i + 1] if i < _L - 1 else S[_L - 1]
            for mo in range(2):
                c0 = ((i - 1) * 2 + mo) * 128
                bias[c0:c0 + 128] = b[i, r * 256 + mo * 128:
                                      r * 256 + (mo + 1) * 128] / s
        for mo in range(16):
            c0 = (28 + mo) * 128
            bias[c0:c0 + 128] = b[0, mo * 128:(mo + 1) * 128] / S[1]
        bias[_ONES_COL] = 1.0
        in_maps.append({"wt0": wt0, "wt": wt_r, "x0": x0, "ident": identity,
                        "bias_mm": bias.reshape(1, -1).astype(np.float16)})
    return in_maps


def kernel(x, W, b, _trace=False):
    from concourse.bass_utils import run_bass_kernel_spmd

    key = "nc"
    if key not in _CACHE:
        _CACHE[key] = _build()
    nc = _CACHE[key]

    in_maps = _prep_inputs(x, W, b)
    res = run_bass_kernel_spmd(
        nc, in_maps, core_ids=list(range(_M)), trace=_trace)
    _CACHE["last_results"] = res
    return np.concatenate(
        [res.results[r]["out"].T.reshape(_SH) for r in range(_M)])


# revision 10
# speedup vs baseline: 1.1903x; 1.1564x over previous
"""Trainium2 Bass kernel for a 16-layer fully-connected chain (matvec per layer).

Computation (reference):
    v = x                       # [2048]
    for i in 0..13:  v = silu(W[i] @ v + b[i])
    out = W[14] @ v + b[14]

Design (8 NeuronCores, one trn2 chip):
  - Layer 0 is computed REDUNDANTLY in full on every core (256 matmul pairs,
    ~7 us, and its 8 MB fp16 weight DMA) -- both hide entirely inside the
    ~50 us ncfw collective-init barrier that stalls the first AllGather
    anyway, eliminating one gather round.
  - Layers 1..14 are row-sharded: core r computes output neurons
    [r*256, (r+1)*256) as two m-tiles of 128, outputs living ACROSS the 128
    partitions (weights-stationary matmuls: lhsT = W^T tile [k=128, m=128],
    rhs = activation column [128,1], fp32 PSUM accumulated over 16 k-tiles).
    Biases are folded into PSUM with a rank-1 matmul (lhsT = bias row
    [1,128], rhs = const 1.0), so the epilogue is one [128,2] sigmoid + one
    tensor-multiply.
  - fp16 weights+activations with per-layer power-of-2 activation scaling
    folded into the weights host-side (activations grow ~3.2x/layer to ~3e7,
    far beyond fp16 range): psum = y_i/S[i+1], sg = sigmoid(S[i+1]*psum),
    v' = psum*sg (silu is not scale-equivariant). Final layer:
    out = S[14]*psum in fp32.
  - All weight slices are DMA'd to SBUF up front and stream behind compute.
  - Inter-layer gather: the [128, 2] half-pair is PE-transposed to [2, 128]
    so every comm DMA is per-partition contiguous (a [128, few-bytes] DMA
    shatters into per-partition 4B packets, ~13 us/layer of queue drain);
    one fp16 AllGather (512 B -> 4 KB) per layer through internal DRAM
    bounce buffers, then a [16,128] -> [128,16] PE transpose back.
"""

import numpy as np

_L = 15        # number of weight matrices
_N = 2048      # neurons per layer
_M = 8         # cores
_SH = _N // _M  # 256 outputs per core = 2 m-tiles of 128

# S[i] = scale of the stored activation entering layer i (v_i = S[i]*v'_i).
_S = [1.0, 0.5, 1.0, 4.0, 8.0, 32.0, 128.0, 256.0, 1024.0, 2048.0,
      8192.0, 16384.0, 65536.0, 262144.0, 524288.0]

# bias_mm column bases (x128): layers 1..14 pairs first, then layer-0 full.
_NBIAS = 28 + 16
_ONES_COL = _NBIAS * 128

_CACHE = {}


def _build():
    import concourse.bacc as bacc
    import concourse.mybir as mybir
    import concourse.tile as tile

    f32 = mybir.dt.float32
    f16 = mybir.dt.float16
    AF = mybir.ActivationFunctionType

    nc = bacc.Bacc("TRN2", target_bir_lowering=False, debug=False,
                   num_devices=_M)

    # full layer-0 weights: col (kt*16 + mo)*128 + pm holds
    # 2*W[0][mo*128+pm, kt*128+pk] at partition pk.
    wt0 = nc.dram_tensor("wt0", [128, 256 * 128], f16, kind="ExternalInput")
    # sharded layers 1..14: per layer, col ((mt*8 + j)*2 + mo)*128 + pm holds
    # W[i][r*256+mo*128+pm, j*256+mt*128+pk] at partition pk (pre-scaled).
    wt = nc.dram_tensor("wt", [_L - 1, 128, 32 * 128], f16,
                        kind="ExternalInput")
    ident = nc.dram_tensor("ident", [128, 128], f16, kind="ExternalInput")
    x0 = nc.dram_tensor("x0", [128, 16], f16, kind="ExternalInput")
    bias_mm = nc.dram_tensor("bias_mm", [1, _NBIAS * 128 + 1], f16,
                             kind="ExternalInput")
    out = nc.dram_tensor("out", [128, 2], f32, kind="ExternalOutput")

    with tile.TileContext(nc) as tc:
        with (
            tc.tile_pool(name="w0", bufs=1) as w0pool,
            tc.tile_pool(name="w", bufs=_L - 1) as wpool,
            tc.tile_pool(name="g", bufs=_L) as gpool,
            tc.tile_pool(name="src", bufs=4) as srcpool,
            tc.tile_pool(name="sg", bufs=4) as sgpool,
            tc.tile_pool(name="c", bufs=1) as cpool,
            tc.tile_pool(name="ps", bufs=3, space="PSUM") as pspool,
            tc.tile_pool(name="psf", bufs=1, space="PSUM") as psfpool,
            tc.tile_pool(name="pst", bufs=2, space="PSUM") as pstpool,
            tc.tile_pool(name="pgt", bufs=2, space="PSUM") as pgtpool,
            tc.tile_pool(name="dram", bufs=2 * _L, space="DRAM") as dpool,
        ):
            bias_t = cpool.tile([1, _NBIAS * 128 + 1], f16)
            nc.sync.dma_start(bias_t[:], bias_mm.ap())
            ident_t = cpool.tile([128, 128], f16)
            nc.sync.dma_start(ident_t[:], ident.ap())

            gath = [gpool.tile([128, 16], f16, tag="g", name=f"gath{i}")
                    for i in range(_L)]
            nc.sync.dma_start(gath[0][:], x0.ap())

            w0sb = w0pool.tile([128, 256 * 128], f16)
            nc.sync.dma_start(w0sb[:], wt0.ap())
            # Throwaway AllGather issued immediately: it absorbs the ncfw
            # init barrier + first-collective warmup (~12 us) so the first
            # REAL gather runs at steady-state cost. Output is never read.
            warm_in = dpool.tile([2, 128], f16, tag="warmin")
            nc.scalar.dma_start(warm_in[:], ident_t[:2, :])
            warm_out = dpool.tile([16, 128], f16, tag="warmout")
            nc.gpsimd.collective_compute(
                "AllGather",
                mybir.AluOpType.bypass,
                replica_groups=[list(range(_M))],
                ins=[warm_in.opt()],
                outs=[warm_out.opt()],
            )

            wsb = [None]
            for i in range(1, _L):
                w = wpool.tile([128, 32 * 128], f16, tag="w")
                nc.sync.dma_start(w[:], wt.ap()[i - 1])
                wsb.append(w)

            ones = bias_t[:, _ONES_COL:_ONES_COL + 1]

            # ---- layer 0: full 2048x2048 matvec on every core ----
            # One start/stop pair per 2KB PSUM zero region: start=True zeroes
            # the WHOLE region, so only the very first matmul starts and only
            # the very last stops; per-address first-touch semantics handle
            # the other columns.
            psf = psfpool.tile([128, 16], f32)
            for kt in range(16):
                for mo in range(16):
                    c = kt * 16 + mo
                    nc.tensor.matmul(
                        psf[:, mo:mo + 1],
                        lhsT=w0sb[:, c * 128:(c + 1) * 128],
                        rhs=gath[0][:, kt:kt + 1],
                        start=(kt == 0 and mo == 0), stop=False,
                    )
            for mo in range(16):
                c0 = (28 + mo) * 128
                nc.tensor.matmul(
                    psf[:, mo:mo + 1],
                    lhsT=bias_t[:, c0:c0 + 128],
                    rhs=ones,
                    start=False, stop=(mo == 15),
                )
            sgf = sgpool.tile([128, 16], f32, tag="sgf")
            nc.scalar.activation(sgf[:], psf[:], AF.Sigmoid,
                                 scale=float(_S[1]))
            nc.vector.tensor_mul(gath[1][:], psf[:], sgf[:])

            # ---- layers 1..14: row-sharded with AllGather between ----
            for i in range(1, _L):
                ps = pspool.tile([128, 2], f32, tag="ps", name=f"ps{i}")
                for mt in range(2):
                    for j in range(_M):
                        for mo in range(2):
                            c = (mt * 8 + j) * 2 + mo
                            nc.tensor.matmul(
                                ps[:, mo:mo + 1],
                                lhsT=wsb[i][:, c * 128:(c + 1) * 128],
                                rhs=gath[i][:, 2 * j + mt:2 * j + mt + 1],
                                start=(mt == 0 and j == 0 and mo == 0),
                                stop=False,
                            )
                for mo in range(2):
                    c0 = ((i - 1) * 2 + mo) * 128
                    nc.tensor.matmul(
                        ps[:, mo:mo + 1],
                        lhsT=bias_t[:, c0:c0 + 128],
                        rhs=ones,
                        start=False, stop=(mo == 1),
                    )
                if i < _L - 1:
                    src = srcpool.tile([128, 2], f16, tag="src")
                    sg = sgpool.tile([128, 2], f32, tag="sg")
                    # psum = y_i/S[i+1] (bias folded); sg = sigmoid(y_i)
                    nc.scalar.activation(sg[:], ps[:], AF.Sigmoid,
                                         scale=float(_S[i + 1]))
                    nc.vector.tensor_mul(src[:], ps[:], sg[:])
                    # PE-transpose [128,2] -> [2,128] so comm DMAs stay
                    # per-partition contiguous.
                    srcT = pstpool.tile([2, 128], f16, tag="pst",
                                        name=f"srcT{i}")
                    nc.tensor.transpose(srcT[:], src[:], ident_t[:])
                    srcTs = srcpool.tile([2, 128], f16, tag="srcTs")
                    nc.vector.tensor_copy(srcTs[:], srcT[:])
                    cc_in = dpool.tile([2, 128], f16, tag="ccin")
                    nc.scalar.dma_start(cc_in[:], srcTs[:])
                    cc_out = dpool.tile([16, 128], f16, tag="ccout")
                    nc.gpsimd.collective_compute(
                        "AllGather",
                        mybir.AluOpType.bypass,
                        replica_groups=[list(range(_M))],
                        ins=[cc_in.opt()],
                        outs=[cc_out.opt()],
                    )
                    rdraw = srcpool.tile([16, 128], f16, tag="rdraw")
                    nc.scalar.dma_start(rdraw[:], cc_out[:])
                    gT = pgtpool.tile([128, 16], f16, tag="gT",
                                      name=f"gT{i}")
                    nc.tensor.transpose(gT[:], rdraw[:], ident_t[:16, :16])
                    nc.vector.tensor_copy(gath[i + 1][:], gT[:])
                else:
                    o = srcpool.tile([128, 2], f32, tag="o")
                    # out = S[14] * psum  (bias already folded into psum)
                    nc.scalar.activation(o[:], ps[:], AF.Identity,
                                         scale=float(_S[14]))
                    nc.sync.dma_start(out.ap(), o[:])

    nc.compile()
    return nc


def _prep_inputs(x, W, b):
    """Host-side scaling, transposition and per-core slicing."""
    x = np.asarray(x, np.float32)
    W = np.asarray(W, np.float32)
    b = np.asarray(b, np.float32)
    S = _S

    Wf = np.empty_like(W)
    for i in range(_L - 1):
        Wf[i] = W[i] * (S[i] / S[i + 1])
    Wf[_L - 1] = W[_L - 1]  # folded with S15 = S14

    # layer-0 full: [pk, kt, mo, pm] -> col (kt*16+mo)*128+pm
    W0 = Wf[0].reshape(16, 128, 16, 128)        # [mo, pm, kt, pk]
    wt0 = np.ascontiguousarray(
        W0.transpose(3, 2, 0, 1).reshape(128, 256 * 128)).astype(np.float16)

    # layers 1..14 sharded: Wv[i, rm, mo, pm, ks, mt, pk]
    Wv = Wf[1:].reshape(_L - 1, _M, 2, 128, _M, 2, 128)
    xv = x.reshape(_M, 2, 128)
    # x0[pk, 2j+mt] = x[j*256 + mt*128 + pk]
    x0 = np.ascontiguousarray(
        xv.transpose(2, 0, 1).reshape(128, 16)).astype(np.float16)
    identity = np.eye(128, dtype=np.float16)

    in_maps = []
    for r in range(_M):
        Wc = Wv[:, r]                           # [i, mo, pm, j, mt, pk]
        Wc = Wc.transpose(0, 5, 4, 3, 1, 2)     # [i, pk, mt, j, mo, pm]
        wt_r = np.ascontiguousarray(
            Wc.reshape(_L - 1, 128, 32 * 128)).astype(np.float16)
        # bias_mm: layers 1..14 pairs, then layer-0 full, then 1.0
        bias = np.zeros(_NBIAS * 128 + 1, np.float32)
        for i in range(1, _L):
            s = S[i + 1] if i < _L - 1 else S[_L - 1]
            for mo in range(2):
                c0 = ((i - 1) * 2 + mo) * 128
                bias[c0:c0 + 128] = b[i, r * 256 + mo * 128:
                                      r * 256 + (mo + 1) * 128] / s
        for mo in range(16):
            c0 = (28 + mo) * 128
            bias[c0:c0 + 128] = b[0, mo * 128:(mo + 1) * 128] / S[1]
        bias[_ONES_COL] = 1.0
        in_maps.append({"wt0": wt0, "wt": wt_r, "x0": x0, "ident": identity,
                        "bias_mm": bias.reshape(1, -1).astype(np.float16)})
    return in_maps


def kernel(x, W, b, _trace=False):
    from concourse.bass_utils import run_bass_kernel_spmd

    key = "nc"
    if key not in _CACHE:
        _CACHE[key] = _build()
    nc = _CACHE[key]

    in_maps = _prep_inputs(x, W, b)
    res = run_bass_kernel_spmd(
        nc, in_maps, core_ids=list(range(_M)), trace=_trace)
    _CACHE["last_results"] = res
    return np.concatenate(
        [res.results[r]["out"].T.reshape(_SH) for r in range(_M)])
